# revision 31
# baseline (speedup 1.0000x reference)
"""ArcMarginProduct (subcenter + inter-topk) Trainium2 kernel.

Math note: the reference uses mp=0.0, so phi_mp = cos*cos(0) + sine*sin(0)
== cos bitwise. The inter-topk term therefore cancels exactly:
    one_hot*phi + tk*phi_mp + (1-one_hot-tk)*cos == one_hot*phi + (1-one_hot)*cos
The kernel computes, per row r and class c:
    out[r, c] = 32 * max(cosine[r, 3c:3c+3])            for c != label[r]
The label column is patched on the host: the host already gathers the 3
candidate f32 values per row to stage them, and computes
out[r, l] = 32 * phi(max of the 3) directly (1024 values, postprocess).

Sharding: batch dim across 8 NeuronCores (128 rows/core = SBUF partitions).

Kernel structure (v18) — engine-balanced u8 streaming:
 - Everything moves as u8 (q = round(255*x); monotone, so max commutes;
   host dequantizes with one fused multiply).  Measured machine limits
   that shape the design:
     * DVE 2x perf mode needs EVERY operand 2-byte packed; any u8
       operand forces 1x (~1.0-1.3 ns/col vs ~0.6 at 2x).
     * total DMA ~360-430 GB/s/core across ALL queues, billed on the
       LARGER side of each transfer (so SWDGE u8->bf16 cast-loads pay
       2 B/elem); a single HWDGE queue sustains only ~300 GB/s.
     * ACT casts u8<->bf16 at ~0.92 ns/col; the GPSIMD CAST op exists
       but steals SBUF bandwidth from DVE (~3x slowdown) - unused.
 - Host stages one buffer with per-chunk route-specific layout, one
   HWDGE load per chunk (bulk loads alternate sync/sync/scalar queues).
   Per-chunk routes balance DVE, ACT, and the DMA bus:
     'L': host-packed lexicographic u16 pairs A=(q0<<8)|q2,
          B=(q1<<8)|q2; one u16 TT max at 2x gives (max(q0,q1)<<8)|q2,
          one strided-u8 byte-pair max finishes.  No casts, no SWDGE,
          fully self-contained -> used at the ramp and the tail.
     'a': ACT casts p2 u8->bf16; DVE op1 max(p0u8,p1u8)->bf16 (1x),
          op2 bf16 (2x); ACT casts the result back to u8 (exact:
          integers <= 255); HWDGE store.
     's': 'a' but the bf16 result is SWDGE cast-stored (saves ACT).
     'P': p0,p1 SWDGE cast-loaded u8->bf16 so op1 runs at 2x (DVE
          relief); ACT casts p2 in and the result out; HWDGE store.
     'p': 'P' with a SWDGE cast-store instead of the ACT out-cast.
     'u'/'q'/'g': spare routes for tuning (plain u8 chain / DVE u8 out /
          GPSIMD cast).
 - All input tiles are SBUF-resident (named) so load queues never stall
   on compute; stores are emitted after all loads on the sync queue.
   ACT out-casts run in DVE-completion order to avoid pool-recycle
   deadlocks.  SWDGE cast-loads lead the gpsimd queue; cast-stores
   trail it.
Quantization rel err ~1.5e-3 (2e-2 gate).
"""

import math
import os
import sys

import numpy as np

if "/opt/trn_rl_repo" not in sys.path:
    sys.path.insert(0, "/opt/trn_rl_repo")

import concourse.bass as bass
import concourse.bacc as bacc
import concourse.mybir as mybir
from concourse.bass_utils import run_bass_kernel_spmd
from concourse.tile import TileContext

B = 1024
C = 20000          # out_features
K = 3              # subcenters
CK = C * K         # 60000
NCORES = 8
RB = B // NCORES   # 128 rows per core

# chunk plan: "<width><route>,...", see module docstring for routes
_PLAN = os.environ.get(
    "V_PLAN", "600L,1500p,2000s,2500P,2000a,2500p,2000s,2400P,2000a,1500p,1000L"
)
PLAN = [(int(t[:-1]), t[-1]) for t in _PLAN.split(",")]
assert sum(w for w, _ in PLAN) == C, PLAN
# byte offset of each chunk in the staged buffer (L chunks use 4 B/col,
# others 3 B/col)
BYTEOFF = []
_o = 0
for _w, _r in PLAN:
    BYTEOFF.append(_o)
    _o += 4 * _w if _r == "L" else 3 * _w
PALL_BYTES = _o
CHOFF = []  # column offset of each chunk
_o = 0
for _w, _r in PLAN:
    CHOFF.append(_o)
    _o += _w

SCALE = 32.0
MARGIN = 0.2
COS_M = math.cos(MARGIN)
SIN_M = math.sin(MARGIN)
TH = math.cos(math.pi - MARGIN)
MMM = 1.0 + math.cos(math.pi - MARGIN)

_CACHED_NC = None


def build():
    u8 = mybir.dt.uint8
    u16 = mybir.dt.uint16
    bf16 = mybir.dt.bfloat16
    Act = mybir.ActivationFunctionType

    nc = bacc.Bacc()
    pall_d = nc.declare_dram_parameter("pall", [RB, PALL_BYTES], u8, isOutput=False)
    out_d = nc.declare_dram_parameter("out", [RB, C], u8, isOutput=True)

    def wmax_of(routes):
        return max([w for w, r in PLAN if r in routes], default=1)

    with TileContext(nc) as tc:
        with (
            tc.tile_pool(name="bfin", bufs=1) as bfpool,
            tc.tile_pool(name="inp", bufs=3) as ipool,
            tc.tile_pool(name="lexp", bufs=2) as lpool,
            tc.tile_pool(name="mid", bufs=2) as mpool,
            tc.tile_pool(name="cast", bufs=4) as cpool,
            tc.tile_pool(name="outp", bufs=3) as opool,
            tc.tile_pool(name="pres", bufs=3) as prpool,
            tc.tile_pool(name="t01g", bufs=1) as gpool,
        ):
            # ---- gpsimd queue, part 1: SWDGE cast-loads for P/p chunks.
            # Only the first V_NEARLY are issued upfront: issuing all of
            # them floods the early DMA bus (SWDGE pays the 2B side) and
            # starves the HWDGE bulk loads DVE needs first.  The rest are
            # chained behind s/p cast-stores inside the compute loop.
            NEARLY = int(os.environ.get("V_NEARLY", "2"))
            bfin = {}
            swdge_pend = []   # chunk ids whose cast-load is deferred
            bfin_loaded = set()

            def emit_bfin_load(ci):
                w = PLAN[ci][0]
                nc.gpsimd.dma_start(
                    out=bfin[ci][:],
                    in_=pall_d[:, BYTEOFF[ci] : BYTEOFF[ci] + 2 * w],
                )
                bfin_loaded.add(ci)

            for ci, (w, route) in enumerate(PLAN):
                if route in "Ppq":
                    bfin[ci] = bfpool.tile([RB, 2 * w], bf16, name=f"bfin_{ci}")
                    if len(bfin_loaded) < NEARLY:
                        emit_bfin_load(ci)
                    else:
                        swdge_pend.append(ci)

            # ---- input loads: bulk chunks alternate between the sync
            # and vector HWDGE queues (one queue sustains only ~300 GB/s);
            # small p2-only loads ride the scalar queue.
            intile = {}
            bulk_q = [{"s": nc.sync, "c": nc.scalar}[c] for c in os.environ.get("V_QPAT", "ssc")]
            bi = 0
            for ci, (w, route) in enumerate(PLAN):
                if route == "L":
                    t = bfpool.tile([RB, 4 * w], u8, name=f"lexin_{ci}")
                    bulk_q[bi % len(bulk_q)].dma_start(
                        out=t[:], in_=pall_d[:, BYTEOFF[ci] : BYTEOFF[ci] + 4 * w]
                    )
                    bi += 1
                elif route in "Ppq":
                    t = bfpool.tile([RB, w], u8, name=f"p2in_{ci}")
                    nc.sync.dma_start(
                        out=t[:],
                        in_=pall_d[:, BYTEOFF[ci] + 2 * w : BYTEOFF[ci] + 3 * w],
                    )
                else:  # u/a/g/s: full 3-plane chunk
                    t = bfpool.tile([RB, 3 * w], u8, name=f"in3_{ci}")
                    bulk_q[bi % len(bulk_q)].dma_start(
                        out=t[:], in_=pall_d[:, BYTEOFF[ci] : BYTEOFF[ci] + 3 * w]
                    )
                    bi += 1
                intile[ci] = t

            def planes(ci):
                w, route = PLAN[ci]
                t = intile[ci]
                if route in "Ppq":
                    return None, None, t[:, :w]
                return t[:, :w], t[:, w : 2 * w], t[:, 2 * w : 3 * w]

            # ---- software-pipelined per-chunk emission: in-cast (ACT),
            # op1+op2 (DVE) in chunk order; ACT out-casts lag OLAG chunks
            # behind so the ACT stream interleaves in- and out-casts
            # instead of serializing all out-casts after all in-casts.
            assert not any(r == "g" for _, r in PLAN), "g route retired"
            OLAG = int(os.environ.get("V_OLAG", "2"))
            dvecast = {
                int(x) for x in os.environ.get("V_DVECAST", "").split(",") if x
            }
            bres = {}      # ci -> bf16 result tile for SWDGE cast-stores
            pending = []   # (ci, bf16 tile) awaiting ACT out-cast + store

            def emit_outcast(ci, o):
                w = PLAN[ci][0]
                o8 = opool.tile([RB, wmax_of("aPg")], u8, tag="out8")
                nc.scalar.activation(o8[:, :w], o[:, :w], Act.Identity)
                nc.sync.dma_start(
                    out=out_d[:, CHOFF[ci] : CHOFF[ci] + w], in_=o8[:, :w]
                )

            def emit_store(ci, o):
                w = PLAN[ci][0]
                nc.sync.dma_start(
                    out=out_d[:, CHOFF[ci] : CHOFF[ci] + w], in_=o[:, :w]
                )

            for ci, (w, route) in enumerate(PLAN):
                if route in "asPpq":
                    _, _, p2 = planes(ci)
                    t = cpool.tile([RB, wmax_of("aPpgs")], bf16, tag="p2b")
                    if ci in dvecast:
                        nc.vector.tensor_copy(t[:, :w], p2)
                    else:
                        nc.scalar.activation(t[:, :w], p2, Act.Identity)
                    p2b = t

                if route == "L":
                    lex = intile[ci]
                    A = lex[:, : 2 * w].bitcast(u16)
                    Bv = lex[:, 2 * w :].bitcast(u16)
                    r = mpool.tile([RB, wmax_of("L")], u16, tag="lexr")
                    nc.vector.tensor_max(r[:, :w], A, Bv)
                    r3 = r[:, :w].bitcast(u8).rearrange("p (w k) -> p w k", k=2)
                    o = opool.tile([RB, wmax_of("L")], u8, tag="outL")
                    nc.vector.tensor_max(o[:, :w], r3[:, :, 1], r3[:, :, 0])
                    emit_store(ci, o)
                elif route == "u":
                    p0, p1, p2 = planes(ci)
                    t = mpool.tile([RB, wmax_of("u")], u8, tag="t01u")
                    nc.vector.tensor_max(t[:, :w], p0, p1)
                    o = opool.tile([RB, wmax_of("u")], u8, tag="outu")
                    nc.vector.tensor_max(o[:, :w], t[:, :w], p2)
                    emit_store(ci, o)
                elif route in "as":
                    p0, p1, _ = planes(ci)
                    t = mpool.tile([RB, wmax_of("aPpgs")], bf16, tag="t01b")
                    nc.vector.tensor_max(t[:, :w], p0, p1)
                    if route == "a":
                        o = mpool.tile([RB, wmax_of("aPg")], bf16, tag="ob")
                        nc.vector.tensor_max(o[:, :w], t[:, :w], p2b[:, :w])
                        pending.append((ci, o))
                    else:
                        o = prpool.tile([RB, wmax_of("sp")], bf16, tag="sres")
                        nc.vector.tensor_max(o[:, :w], t[:, :w], p2b[:, :w])
                        bres[ci] = o
                elif route in "Ppq":
                    if ci not in bfin_loaded:
                        emit_bfin_load(ci)
                        swdge_pend.remove(ci)
                    bt = bfin[ci]
                    t = mpool.tile([RB, wmax_of("aPpgs")], bf16, tag="t01b")
                    nc.vector.tensor_max(t[:, :w], bt[:, :w], bt[:, w : 2 * w])
                    if route == "q":
                        o = opool.tile([RB, wmax_of("q")], u8, tag="outq")
                        nc.vector.tensor_max(o[:, :w], t[:, :w], p2b[:, :w])
                        emit_store(ci, o)
                    elif route == "P":
                        o = mpool.tile([RB, wmax_of("aPg")], bf16, tag="ob")
                        nc.vector.tensor_max(o[:, :w], t[:, :w], p2b[:, :w])
                        pending.append((ci, o))
                    else:
                        o = prpool.tile([RB, wmax_of("sp")], bf16, tag="sres")
                        nc.vector.tensor_max(o[:, :w], t[:, :w], p2b[:, :w])
                        bres[ci] = o

                # s/p chunks: the SWDGE cast-store rides the gpsimd queue
                # right after this chunk's op2, and gates the next deferred
                # cast-load (naturally staggering SWDGE bus usage)
                if route in "sp":
                    nc.gpsimd.dma_start(
                        out=out_d[:, CHOFF[ci] : CHOFF[ci] + w],
                        in_=bres[ci][:, :w],
                    )
                    if swdge_pend:
                        nxt = swdge_pend.pop(0)
                        emit_bfin_load(nxt)

                while len(pending) > OLAG:
                    emit_outcast(*pending.pop(0))

            for x in pending:
                emit_outcast(*x)

    nc.finalize()
    return nc


def _make_in_maps(cosine: np.ndarray, label: np.ndarray):
    # uint8 staging: q = round(255*x). x in [0,1) so 255*x+0.5 in [0.5,255.5)
    # and the float->int truncation implements round-half-up exactly.
    q = (cosine * np.float32(255.0) + np.float32(0.5)).astype(np.uint8)
    q3 = q.reshape(B, C, K)
    pall = np.empty((B, PALL_BYTES), dtype=np.uint8)
    for (w, route), bo, co in zip(PLAN, BYTEOFF, CHOFF):
        blk = q3[:, co : co + w, :]
        if route == "L":
            # A = (q0<<8)|q2, B = (q1<<8)|q2 little-endian: bytes [q2, qk]
            pall[:, bo : bo + 2 * w : 2] = blk[:, :, 2]
            pall[:, bo + 1 : bo + 2 * w : 2] = blk[:, :, 0]
            pall[:, bo + 2 * w : bo + 4 * w : 2] = blk[:, :, 2]
            pall[:, bo + 2 * w + 1 : bo + 4 * w : 2] = blk[:, :, 1]
        else:
            for k in range(K):
                pall[:, bo + k * w : bo + (k + 1) * w] = blk[:, :, k]
    in_maps = []
    for i in range(NCORES):
        rs = slice(i * RB, (i + 1) * RB)
        in_maps.append({"pall": np.ascontiguousarray(pall[rs])})
    return in_maps


def _postprocess(per_core_outs, cosine: np.ndarray, label: np.ndarray) -> np.ndarray:
    out_q = np.concatenate([np.asarray(o) for o in per_core_outs], axis=0)
    # dequantize + the *32 scale in one fused host multiply
    out = out_q.astype(np.float32) * np.float32(SCALE / 255.0)
    # label column: exact phi from the full-precision gathered candidates
    lab = np.asarray(label, dtype=np.int64)
    rows = np.arange(B)
    idx = (3 * lab)[:, None] + np.arange(K)[None, :]
    g3 = np.asarray(cosine, dtype=np.float32)[rows[:, None], idx]
    cl = g3.max(axis=1)
    sine = np.sqrt(np.maximum(np.float32(1.0) - cl * cl, np.float32(0.0)))
    phi = cl * np.float32(COS_M) - sine * np.float32(SIN_M)
    phi = np.where(cl > np.float32(TH), phi, cl - np.float32(MMM))
    out[rows, lab] = np.float32(SCALE) * phi.astype(np.float32)
    return np.ascontiguousarray(out)


def kernel(cosine: np.ndarray, label: np.ndarray) -> np.ndarray:
    global _CACHED_NC
    cosine = np.asarray(cosine)
    label = np.asarray(label)
    assert cosine.shape == (B, CK), cosine.shape
    assert label.shape == (B,), label.shape

    if _CACHED_NC is None:
        _CACHED_NC = build()
    nc = _CACHED_NC

    in_maps = _make_in_maps(cosine, label)
    res = run_bass_kernel_spmd(nc, in_maps, core_ids=list(range(NCORES)))
    return _postprocess(
        [res.results[i]["out"] for i in range(NCORES)], cosine, label
    )


# revision 32
# speedup vs baseline: 1.0227x; 1.0227x over previous
"""ArcMarginProduct (subcenter + inter-topk) Trainium2 kernel.

Math note: the reference uses mp=0.0, so phi_mp = cos*cos(0) + sine*sin(0)
== cos bitwise. The inter-topk term therefore cancels exactly:
    one_hot*phi + tk*phi_mp + (1-one_hot-tk)*cos == one_hot*phi + (1-one_hot)*cos
The kernel computes, per row r and class c:
    out[r, c] = 32 * max(cosine[r, 3c:3c+3])            for c != label[r]
The label column is patched on the host: the host already gathers the 3
candidate f32 values per row to stage them, and computes
out[r, l] = 32 * phi(max of the 3) directly (1024 values, postprocess).

Sharding: batch dim across 8 NeuronCores (128 rows/core = SBUF partitions).

Kernel structure (v18) — engine-balanced u8 streaming:
 - Everything moves as u8 (q = round(255*x); monotone, so max commutes;
   host dequantizes with one fused multiply).  Measured machine limits
   that shape the design:
     * DVE 2x perf mode needs EVERY operand 2-byte packed; any u8
       operand forces 1x (~1.0-1.3 ns/col vs ~0.6 at 2x).
     * total DMA ~360-430 GB/s/core across ALL queues, billed on the
       LARGER side of each transfer (so SWDGE u8->bf16 cast-loads pay
       2 B/elem); a single HWDGE queue sustains only ~300 GB/s.
     * ACT casts u8<->bf16 at ~0.92 ns/col; the GPSIMD CAST op exists
       but steals SBUF bandwidth from DVE (~3x slowdown) - unused.
 - Host stages one buffer with per-chunk route-specific layout, one
   HWDGE load per chunk (bulk loads alternate sync/sync/scalar queues).
   Per-chunk routes balance DVE, ACT, and the DMA bus:
     'L': host-packed lexicographic u16 pairs A=(q0<<8)|q2,
          B=(q1<<8)|q2; one u16 TT max at 2x gives (max(q0,q1)<<8)|q2,
          one strided-u8 byte-pair max finishes.  No casts, no SWDGE,
          fully self-contained -> used at the ramp and the tail.
     'a': ACT casts p2 u8->bf16; DVE op1 max(p0u8,p1u8)->bf16 (1x),
          op2 bf16 (2x); ACT casts the result back to u8 (exact:
          integers <= 255); HWDGE store.
     's': 'a' but the bf16 result is SWDGE cast-stored (saves ACT).
     'P': p0,p1 SWDGE cast-loaded u8->bf16 so op1 runs at 2x (DVE
          relief); ACT casts p2 in and the result out; HWDGE store.
     'p': 'P' with a SWDGE cast-store instead of the ACT out-cast.
     'u'/'q'/'g': spare routes for tuning (plain u8 chain / DVE u8 out /
          GPSIMD cast).
 - All input tiles are SBUF-resident (named) so load queues never stall
   on compute; stores are emitted after all loads on the sync queue.
   ACT out-casts run in DVE-completion order to avoid pool-recycle
   deadlocks.  SWDGE cast-loads lead the gpsimd queue; cast-stores
   trail it.
Quantization rel err ~1.5e-3 (2e-2 gate).
"""

import math
import os
import sys

import numpy as np

if "/opt/trn_rl_repo" not in sys.path:
    sys.path.insert(0, "/opt/trn_rl_repo")

import concourse.bass as bass
import concourse.bacc as bacc
import concourse.mybir as mybir
from concourse.bass_utils import run_bass_kernel_spmd
from concourse.tile import TileContext

B = 1024
C = 20000          # out_features
K = 3              # subcenters
CK = C * K         # 60000
NCORES = 8
RB = B // NCORES   # 128 rows per core

# chunk plan: "<width><route>,...", see module docstring for routes
_PLAN = os.environ.get(
    "V_PLAN", "600L,1500p,2000s,2500P,2000a,2500p,2000s,2400P,2000a,1500p,1000L"
)
PLAN = [(int(t[:-1]), t[-1]) for t in _PLAN.split(",")]
assert sum(w for w, _ in PLAN) == C, PLAN
# byte offset of each chunk in the staged buffer (L chunks use 4 B/col,
# others 3 B/col)
BYTEOFF = []
_o = 0
for _w, _r in PLAN:
    BYTEOFF.append(_o)
    _o += 4 * _w if _r == "L" else 3 * _w
PALL_BYTES = _o
CHOFF = []  # column offset of each chunk
_o = 0
for _w, _r in PLAN:
    CHOFF.append(_o)
    _o += _w

SCALE = 32.0
MARGIN = 0.2
COS_M = math.cos(MARGIN)
SIN_M = math.sin(MARGIN)
TH = math.cos(math.pi - MARGIN)
MMM = 1.0 + math.cos(math.pi - MARGIN)

_CACHED_NC = None


def build():
    u8 = mybir.dt.uint8
    u16 = mybir.dt.uint16
    bf16 = mybir.dt.bfloat16
    Act = mybir.ActivationFunctionType

    nc = bacc.Bacc()
    pall_d = nc.declare_dram_parameter("pall", [RB, PALL_BYTES], u8, isOutput=False)
    out_d = nc.declare_dram_parameter("out", [RB, C], u8, isOutput=True)

    def wmax_of(routes):
        return max([w for w, r in PLAN if r in routes], default=1)

    with TileContext(nc) as tc:
        with (
            tc.tile_pool(name="bfin", bufs=1) as bfpool,
            tc.tile_pool(name="inp", bufs=3) as ipool,
            tc.tile_pool(name="lexp", bufs=2) as lpool,
            tc.tile_pool(name="mid", bufs=2) as mpool,
            tc.tile_pool(name="cast", bufs=3) as cpool,
            tc.tile_pool(name="outp", bufs=3) as opool,
            tc.tile_pool(name="pres", bufs=3) as prpool,
            tc.tile_pool(name="t01g", bufs=1) as gpool,
        ):
            # ---- gpsimd queue, part 1: SWDGE cast-loads for P/p chunks.
            # Only the first V_NEARLY are issued upfront: issuing all of
            # them floods the early DMA bus (SWDGE pays the 2B side) and
            # starves the HWDGE bulk loads DVE needs first.  The rest are
            # chained behind s/p cast-stores inside the compute loop.
            NEARLY = int(os.environ.get("V_NEARLY", "3"))
            bfin = {}
            swdge_pend = []   # chunk ids whose cast-load is deferred
            bfin_loaded = set()

            def emit_bfin_load(ci):
                w = PLAN[ci][0]
                nc.gpsimd.dma_start(
                    out=bfin[ci][:],
                    in_=pall_d[:, BYTEOFF[ci] : BYTEOFF[ci] + 2 * w],
                )
                bfin_loaded.add(ci)

            for ci, (w, route) in enumerate(PLAN):
                if route in "Ppq":
                    bfin[ci] = bfpool.tile([RB, 2 * w], bf16, name=f"bfin_{ci}")
                    if len(bfin_loaded) < NEARLY:
                        emit_bfin_load(ci)
                    else:
                        swdge_pend.append(ci)

            # ---- input loads: bulk chunks alternate between the sync
            # and vector HWDGE queues (one queue sustains only ~300 GB/s);
            # small p2-only loads ride the scalar queue.
            intile = {}
            bulk_q = [{"s": nc.sync, "c": nc.scalar}[c] for c in os.environ.get("V_QPAT", "ssc")]
            bi = 0
            for ci, (w, route) in enumerate(PLAN):
                if route == "L":
                    t = bfpool.tile([RB, 4 * w], u8, name=f"lexin_{ci}")
                    bulk_q[bi % len(bulk_q)].dma_start(
                        out=t[:], in_=pall_d[:, BYTEOFF[ci] : BYTEOFF[ci] + 4 * w]
                    )
                    bi += 1
                elif route in "Ppq":
                    t = bfpool.tile([RB, w], u8, name=f"p2in_{ci}")
                    nc.sync.dma_start(
                        out=t[:],
                        in_=pall_d[:, BYTEOFF[ci] + 2 * w : BYTEOFF[ci] + 3 * w],
                    )
                else:  # u/a/g/s: full 3-plane chunk
                    t = bfpool.tile([RB, 3 * w], u8, name=f"in3_{ci}")
                    bulk_q[bi % len(bulk_q)].dma_start(
                        out=t[:], in_=pall_d[:, BYTEOFF[ci] : BYTEOFF[ci] + 3 * w]
                    )
                    bi += 1
                intile[ci] = t

            def planes(ci):
                w, route = PLAN[ci]
                t = intile[ci]
                if route in "Ppq":
                    return None, None, t[:, :w]
                return t[:, :w], t[:, w : 2 * w], t[:, 2 * w : 3 * w]

            # ---- software-pipelined per-chunk emission: in-cast (ACT),
            # op1+op2 (DVE) in chunk order; ACT out-casts lag OLAG chunks
            # behind so the ACT stream interleaves in- and out-casts
            # instead of serializing all out-casts after all in-casts.
            assert not any(r == "g" for _, r in PLAN), "g route retired"
            OLAG = int(os.environ.get("V_OLAG", "2"))
            dvecast = {
                int(x) for x in os.environ.get("V_DVECAST", "").split(",") if x
            }
            bres = {}      # ci -> bf16 result tile for SWDGE cast-stores
            pending = []   # (ci, bf16 tile) awaiting ACT out-cast + store

            def emit_outcast(ci, o):
                w = PLAN[ci][0]
                o8 = opool.tile([RB, wmax_of("aPg")], u8, tag="out8")
                nc.scalar.activation(o8[:, :w], o[:, :w], Act.Identity)
                nc.sync.dma_start(
                    out=out_d[:, CHOFF[ci] : CHOFF[ci] + w], in_=o8[:, :w]
                )

            def emit_store(ci, o):
                w = PLAN[ci][0]
                nc.sync.dma_start(
                    out=out_d[:, CHOFF[ci] : CHOFF[ci] + w], in_=o[:, :w]
                )

            for ci, (w, route) in enumerate(PLAN):
                if route in "asPpq":
                    _, _, p2 = planes(ci)
                    t = cpool.tile([RB, wmax_of("aPpgs")], bf16, tag="p2b")
                    if ci in dvecast:
                        nc.vector.tensor_copy(t[:, :w], p2)
                    else:
                        nc.scalar.activation(t[:, :w], p2, Act.Identity)
                    p2b = t

                if route == "L":
                    lex = intile[ci]
                    A = lex[:, : 2 * w].bitcast(u16)
                    Bv = lex[:, 2 * w :].bitcast(u16)
                    r = mpool.tile([RB, wmax_of("L")], u16, tag="lexr")
                    nc.vector.tensor_max(r[:, :w], A, Bv)
                    r3 = r[:, :w].bitcast(u8).rearrange("p (w k) -> p w k", k=2)
                    o = opool.tile([RB, wmax_of("L")], u8, tag="outL")
                    nc.vector.tensor_max(o[:, :w], r3[:, :, 1], r3[:, :, 0])
                    emit_store(ci, o)
                elif route == "u":
                    p0, p1, p2 = planes(ci)
                    t = mpool.tile([RB, wmax_of("u")], u8, tag="t01u")
                    nc.vector.tensor_max(t[:, :w], p0, p1)
                    o = opool.tile([RB, wmax_of("u")], u8, tag="outu")
                    nc.vector.tensor_max(o[:, :w], t[:, :w], p2)
                    emit_store(ci, o)
                elif route in "as":
                    p0, p1, _ = planes(ci)
                    t = mpool.tile([RB, wmax_of("aPpgs")], bf16, tag="t01b")
                    nc.vector.tensor_max(t[:, :w], p0, p1)
                    if route == "a":
                        o = mpool.tile([RB, wmax_of("aPg")], bf16, tag="ob")
                        nc.vector.tensor_max(o[:, :w], t[:, :w], p2b[:, :w])
                        pending.append((ci, o))
                    else:
                        o = prpool.tile([RB, wmax_of("sp")], bf16, tag="sres")
                        nc.vector.tensor_max(o[:, :w], t[:, :w], p2b[:, :w])
                        bres[ci] = o
                elif route in "Ppq":
                    if ci not in bfin_loaded:
                        emit_bfin_load(ci)
                        swdge_pend.remove(ci)
                    bt = bfin[ci]
                    t = mpool.tile([RB, wmax_of("aPpgs")], bf16, tag="t01b")
                    nc.vector.tensor_max(t[:, :w], bt[:, :w], bt[:, w : 2 * w])
                    if route == "q":
                        o = opool.tile([RB, wmax_of("q")], u8, tag="outq")
                        nc.vector.tensor_max(o[:, :w], t[:, :w], p2b[:, :w])
                        emit_store(ci, o)
                    elif route == "P":
                        o = mpool.tile([RB, wmax_of("aPg")], bf16, tag="ob")
                        nc.vector.tensor_max(o[:, :w], t[:, :w], p2b[:, :w])
                        pending.append((ci, o))
                    else:
                        o = prpool.tile([RB, wmax_of("sp")], bf16, tag="sres")
                        nc.vector.tensor_max(o[:, :w], t[:, :w], p2b[:, :w])
                        bres[ci] = o

                # s/p chunks: the SWDGE cast-store rides the gpsimd queue
                # right after this chunk's op2, and gates the next deferred
                # cast-load (naturally staggering SWDGE bus usage)
                if route in "sp":
                    nc.gpsimd.dma_start(
                        out=out_d[:, CHOFF[ci] : CHOFF[ci] + w],
                        in_=bres[ci][:, :w],
                    )
                    if swdge_pend:
                        nxt = swdge_pend.pop(0)
                        emit_bfin_load(nxt)

                while len(pending) > OLAG:
                    emit_outcast(*pending.pop(0))

            for x in pending:
                emit_outcast(*x)

    nc.finalize()
    return nc


def _make_in_maps(cosine: np.ndarray, label: np.ndarray):
    # uint8 staging: q = round(255*x). x in [0,1) so 255*x+0.5 in [0.5,255.5)
    # and the float->int truncation implements round-half-up exactly.
    q = (cosine * np.float32(255.0) + np.float32(0.5)).astype(np.uint8)
    q3 = q.reshape(B, C, K)
    pall = np.empty((B, PALL_BYTES), dtype=np.uint8)
    for (w, route), bo, co in zip(PLAN, BYTEOFF, CHOFF):
        blk = q3[:, co : co + w, :]
        if route == "L":
            # A = (q0<<8)|q2, B = (q1<<8)|q2 little-endian: bytes [q2, qk]
            pall[:, bo : bo + 2 * w : 2] = blk[:, :, 2]
            pall[:, bo + 1 : bo + 2 * w : 2] = blk[:, :, 0]
            pall[:, bo + 2 * w : bo + 4 * w : 2] = blk[:, :, 2]
            pall[:, bo + 2 * w + 1 : bo + 4 * w : 2] = blk[:, :, 1]
        else:
            for k in range(K):
                pall[:, bo + k * w : bo + (k + 1) * w] = blk[:, :, k]
    in_maps = []
    for i in range(NCORES):
        rs = slice(i * RB, (i + 1) * RB)
        in_maps.append({"pall": np.ascontiguousarray(pall[rs])})
    return in_maps


def _postprocess(per_core_outs, cosine: np.ndarray, label: np.ndarray) -> np.ndarray:
    out_q = np.concatenate([np.asarray(o) for o in per_core_outs], axis=0)
    # dequantize + the *32 scale in one fused host multiply
    out = out_q.astype(np.float32) * np.float32(SCALE / 255.0)
    # label column: exact phi from the full-precision gathered candidates
    lab = np.asarray(label, dtype=np.int64)
    rows = np.arange(B)
    idx = (3 * lab)[:, None] + np.arange(K)[None, :]
    g3 = np.asarray(cosine, dtype=np.float32)[rows[:, None], idx]
    cl = g3.max(axis=1)
    sine = np.sqrt(np.maximum(np.float32(1.0) - cl * cl, np.float32(0.0)))
    phi = cl * np.float32(COS_M) - sine * np.float32(SIN_M)
    phi = np.where(cl > np.float32(TH), phi, cl - np.float32(MMM))
    out[rows, lab] = np.float32(SCALE) * phi.astype(np.float32)
    return np.ascontiguousarray(out)


def kernel(cosine: np.ndarray, label: np.ndarray) -> np.ndarray:
    global _CACHED_NC
    cosine = np.asarray(cosine)
    label = np.asarray(label)
    assert cosine.shape == (B, CK), cosine.shape
    assert label.shape == (B,), label.shape

    if _CACHED_NC is None:
        _CACHED_NC = build()
    nc = _CACHED_NC

    in_maps = _make_in_maps(cosine, label)
    res = run_bass_kernel_spmd(nc, in_maps, core_ids=list(range(NCORES)))
    return _postprocess(
        [res.results[i]["out"] for i in range(NCORES)], cosine, label
    )


# revision 33
# speedup vs baseline: 1.0330x; 1.0100x over previous
"""ArcMarginProduct (subcenter + inter-topk) Trainium2 kernel.

Math note: the reference uses mp=0.0, so phi_mp = cos*cos(0) + sine*sin(0)
== cos bitwise. The inter-topk term therefore cancels exactly:
    one_hot*phi + tk*phi_mp + (1-one_hot-tk)*cos == one_hot*phi + (1-one_hot)*cos
The kernel computes, per row r and class c:
    out[r, c] = 32 * max(cosine[r, 3c:3c+3])            for c != label[r]
The label column is patched on the host: the host already gathers the 3
candidate f32 values per row to stage them, and computes
out[r, l] = 32 * phi(max of the 3) directly (1024 values, postprocess).

Sharding: batch dim across 8 NeuronCores (128 rows/core = SBUF partitions).

Kernel structure (v18) — engine-balanced u8 streaming:
 - Everything moves as u8 (q = round(255*x); monotone, so max commutes;
   host dequantizes with one fused multiply).  Measured machine limits
   that shape the design:
     * DVE 2x perf mode needs EVERY operand 2-byte packed; any u8
       operand forces 1x (~1.0-1.3 ns/col vs ~0.6 at 2x).
     * total DMA ~360-430 GB/s/core across ALL queues, billed on the
       LARGER side of each transfer (so SWDGE u8->bf16 cast-loads pay
       2 B/elem); a single HWDGE queue sustains only ~300 GB/s.
     * ACT casts u8<->bf16 at ~0.92 ns/col; the GPSIMD CAST op exists
       but steals SBUF bandwidth from DVE (~3x slowdown) - unused.
 - Host stages one buffer with per-chunk route-specific layout, one
   HWDGE load per chunk (bulk loads alternate sync/sync/scalar queues).
   Per-chunk routes balance DVE, ACT, and the DMA bus:
     'L': host-packed lexicographic u16 pairs A=(q0<<8)|q2,
          B=(q1<<8)|q2; one u16 TT max at 2x gives (max(q0,q1)<<8)|q2,
          one strided-u8 byte-pair max finishes.  No casts, no SWDGE,
          fully self-contained -> used at the ramp and the tail.
     'a': ACT casts p2 u8->bf16; DVE op1 max(p0u8,p1u8)->bf16 (1x),
          op2 bf16 (2x); ACT casts the result back to u8 (exact:
          integers <= 255); HWDGE store.
     's': 'a' but the bf16 result is SWDGE cast-stored (saves ACT).
     'P': p0,p1 SWDGE cast-loaded u8->bf16 so op1 runs at 2x (DVE
          relief); ACT casts p2 in and the result out; HWDGE store.
     'p': 'P' with a SWDGE cast-store instead of the ACT out-cast.
     'u'/'q'/'g': spare routes for tuning (plain u8 chain / DVE u8 out /
          GPSIMD cast).
 - All input tiles are SBUF-resident (named) so load queues never stall
   on compute.  The compute loop is software-pipelined per chunk: ACT
   out-casts lag OLAG chunks behind the in-casts, and only the first
   V_NEARLY SWDGE cast-loads are issued upfront — the rest are chained
   behind s/p cast-stores on the gpsimd queue, so the 2-byte-billed
   SWDGE traffic stops starving the early HWDGE bulk loads.
Quantization rel err ~1.5e-3 (2e-2 gate).
"""

import math
import os
import sys

import numpy as np

if "/opt/trn_rl_repo" not in sys.path:
    sys.path.insert(0, "/opt/trn_rl_repo")

import concourse.bass as bass
import concourse.bacc as bacc
import concourse.mybir as mybir
from concourse.bass_utils import run_bass_kernel_spmd
from concourse.tile import TileContext

B = 1024
C = 20000          # out_features
K = 3              # subcenters
CK = C * K         # 60000
NCORES = 8
RB = B // NCORES   # 128 rows per core

# chunk plan: "<width><route>,...", see module docstring for routes
_PLAN = os.environ.get(
    "V_PLAN", "600L,1500p,2000s,2500P,2000a,2500p,2000s,2400P,2000a,1500p,1000L"
)
PLAN = [(int(t[:-1]), t[-1]) for t in _PLAN.split(",")]
assert sum(w for w, _ in PLAN) == C, PLAN
# byte offset of each chunk in the staged buffer (L chunks use 4 B/col,
# others 3 B/col)
BYTEOFF = []
_o = 0
for _w, _r in PLAN:
    BYTEOFF.append(_o)
    _o += 4 * _w if _r == "L" else 3 * _w
PALL_BYTES = _o
CHOFF = []  # column offset of each chunk
_o = 0
for _w, _r in PLAN:
    CHOFF.append(_o)
    _o += _w

SCALE = 32.0
MARGIN = 0.2
COS_M = math.cos(MARGIN)
SIN_M = math.sin(MARGIN)
TH = math.cos(math.pi - MARGIN)
MMM = 1.0 + math.cos(math.pi - MARGIN)

_CACHED_NC = None


def build():
    u8 = mybir.dt.uint8
    u16 = mybir.dt.uint16
    bf16 = mybir.dt.bfloat16
    Act = mybir.ActivationFunctionType

    nc = bacc.Bacc()
    pall_d = nc.declare_dram_parameter("pall", [RB, PALL_BYTES], u8, isOutput=False)
    out_d = nc.declare_dram_parameter("out", [RB, C], u8, isOutput=True)

    def wmax_of(routes):
        return max([w for w, r in PLAN if r in routes], default=1)

    with TileContext(nc) as tc:
        with (
            tc.tile_pool(name="bfin", bufs=1) as bfpool,
            tc.tile_pool(name="inp", bufs=3) as ipool,
            tc.tile_pool(name="lexp", bufs=2) as lpool,
            tc.tile_pool(name="mid", bufs=2) as mpool,
            tc.tile_pool(name="cast", bufs=3) as cpool,
            tc.tile_pool(name="outp", bufs=3) as opool,
            tc.tile_pool(name="pres", bufs=3) as prpool,
            tc.tile_pool(name="t01g", bufs=1) as gpool,
        ):
            # ---- gpsimd queue, part 1: SWDGE cast-loads for P/p chunks.
            # Only the first V_NEARLY are issued upfront: issuing all of
            # them floods the early DMA bus (SWDGE pays the 2B side) and
            # starves the HWDGE bulk loads DVE needs first.  The rest are
            # chained behind s/p cast-stores inside the compute loop.
            NEARLY = int(os.environ.get("V_NEARLY", "3"))
            bfin = {}
            swdge_pend = []   # chunk ids whose cast-load is deferred
            bfin_loaded = set()

            def emit_bfin_load(ci):
                w = PLAN[ci][0]
                nc.gpsimd.dma_start(
                    out=bfin[ci][:],
                    in_=pall_d[:, BYTEOFF[ci] : BYTEOFF[ci] + 2 * w],
                )
                bfin_loaded.add(ci)

            for ci, (w, route) in enumerate(PLAN):
                if route in "Ppq":
                    bfin[ci] = bfpool.tile([RB, 2 * w], bf16, name=f"bfin_{ci}")
                    if len(bfin_loaded) < NEARLY:
                        emit_bfin_load(ci)
                    else:
                        swdge_pend.append(ci)

            # ---- input loads: bulk chunks alternate between the sync
            # and vector HWDGE queues (one queue sustains only ~300 GB/s);
            # small p2-only loads ride the scalar queue.
            intile = {}
            bulk_q = [{"s": nc.sync, "c": nc.scalar}[c] for c in os.environ.get("V_QPAT", "ssc")]
            bi = 0
            for ci, (w, route) in enumerate(PLAN):
                if route == "L":
                    t = bfpool.tile([RB, 4 * w], u8, name=f"lexin_{ci}")
                    bulk_q[bi % len(bulk_q)].dma_start(
                        out=t[:], in_=pall_d[:, BYTEOFF[ci] : BYTEOFF[ci] + 4 * w]
                    )
                    bi += 1
                elif route in "Ppq":
                    t = bfpool.tile([RB, w], u8, name=f"p2in_{ci}")
                    nc.sync.dma_start(
                        out=t[:],
                        in_=pall_d[:, BYTEOFF[ci] + 2 * w : BYTEOFF[ci] + 3 * w],
                    )
                else:  # u/a/g/s: full 3-plane chunk
                    t = bfpool.tile([RB, 3 * w], u8, name=f"in3_{ci}")
                    bulk_q[bi % len(bulk_q)].dma_start(
                        out=t[:], in_=pall_d[:, BYTEOFF[ci] : BYTEOFF[ci] + 3 * w]
                    )
                    bi += 1
                intile[ci] = t

            def planes(ci):
                w, route = PLAN[ci]
                t = intile[ci]
                if route in "Ppq":
                    return None, None, t[:, :w]
                return t[:, :w], t[:, w : 2 * w], t[:, 2 * w : 3 * w]

            # ---- software-pipelined per-chunk emission: in-cast (ACT),
            # op1+op2 (DVE) in chunk order; ACT out-casts lag OLAG chunks
            # behind so the ACT stream interleaves in- and out-casts
            # instead of serializing all out-casts after all in-casts.
            assert not any(r == "g" for _, r in PLAN), "g route retired"
            OLAG = int(os.environ.get("V_OLAG", "2"))
            dvecast = {
                int(x) for x in os.environ.get("V_DVECAST", "").split(",") if x
            }
            bres = {}      # ci -> bf16 result tile for SWDGE cast-stores
            pending = []   # (ci, bf16 tile) awaiting ACT out-cast + store

            def emit_outcast(ci, o):
                w = PLAN[ci][0]
                o8 = opool.tile([RB, wmax_of("aPg")], u8, tag="out8")
                nc.scalar.activation(o8[:, :w], o[:, :w], Act.Identity)
                nc.sync.dma_start(
                    out=out_d[:, CHOFF[ci] : CHOFF[ci] + w], in_=o8[:, :w]
                )

            def emit_store(ci, o):
                w = PLAN[ci][0]
                nc.sync.dma_start(
                    out=out_d[:, CHOFF[ci] : CHOFF[ci] + w], in_=o[:, :w]
                )

            for ci, (w, route) in enumerate(PLAN):
                if route in "asPpq":
                    _, _, p2 = planes(ci)
                    t = cpool.tile([RB, wmax_of("aPpgs")], bf16, tag="p2b")
                    if ci in dvecast:
                        nc.vector.tensor_copy(t[:, :w], p2)
                    else:
                        nc.scalar.activation(t[:, :w], p2, Act.Identity)
                    p2b = t

                if route == "L":
                    lex = intile[ci]
                    A = lex[:, : 2 * w].bitcast(u16)
                    Bv = lex[:, 2 * w :].bitcast(u16)
                    r = mpool.tile([RB, wmax_of("L")], u16, tag="lexr")
                    nc.vector.tensor_max(r[:, :w], A, Bv)
                    r3 = r[:, :w].bitcast(u8).rearrange("p (w k) -> p w k", k=2)
                    o = opool.tile([RB, wmax_of("L")], u8, tag="outL")
                    nc.vector.tensor_max(o[:, :w], r3[:, :, 1], r3[:, :, 0])
                    emit_store(ci, o)
                elif route == "u":
                    p0, p1, p2 = planes(ci)
                    t = mpool.tile([RB, wmax_of("u")], u8, tag="t01u")
                    nc.vector.tensor_max(t[:, :w], p0, p1)
                    o = opool.tile([RB, wmax_of("u")], u8, tag="outu")
                    nc.vector.tensor_max(o[:, :w], t[:, :w], p2)
                    emit_store(ci, o)
                elif route in "as":
                    p0, p1, _ = planes(ci)
                    t = mpool.tile([RB, wmax_of("aPpgs")], bf16, tag="t01b")
                    nc.vector.tensor_max(t[:, :w], p0, p1)
                    if route == "a":
                        o = mpool.tile([RB, wmax_of("aPg")], bf16, tag="ob")
                        nc.vector.tensor_max(o[:, :w], t[:, :w], p2b[:, :w])
                        pending.append((ci, o))
                    else:
                        o = prpool.tile([RB, wmax_of("sp")], bf16, tag="sres")
                        nc.vector.tensor_max(o[:, :w], t[:, :w], p2b[:, :w])
                        bres[ci] = o
                elif route in "Ppq":
                    if ci not in bfin_loaded:
                        emit_bfin_load(ci)
                        swdge_pend.remove(ci)
                    bt = bfin[ci]
                    t = mpool.tile([RB, wmax_of("aPpgs")], bf16, tag="t01b")
                    nc.vector.tensor_max(t[:, :w], bt[:, :w], bt[:, w : 2 * w])
                    if route == "q":
                        o = opool.tile([RB, wmax_of("q")], u8, tag="outq")
                        nc.vector.tensor_max(o[:, :w], t[:, :w], p2b[:, :w])
                        emit_store(ci, o)
                    elif route == "P":
                        o = mpool.tile([RB, wmax_of("aPg")], bf16, tag="ob")
                        nc.vector.tensor_max(o[:, :w], t[:, :w], p2b[:, :w])
                        pending.append((ci, o))
                    else:
                        o = prpool.tile([RB, wmax_of("sp")], bf16, tag="sres")
                        nc.vector.tensor_max(o[:, :w], t[:, :w], p2b[:, :w])
                        bres[ci] = o

                # s/p chunks: the SWDGE cast-store rides the gpsimd queue
                # right after this chunk's op2, and gates the next deferred
                # cast-load (naturally staggering SWDGE bus usage)
                if route in "sp":
                    nc.gpsimd.dma_start(
                        out=out_d[:, CHOFF[ci] : CHOFF[ci] + w],
                        in_=bres[ci][:, :w],
                    )
                    if swdge_pend:
                        nxt = swdge_pend.pop(0)
                        emit_bfin_load(nxt)

                while len(pending) > OLAG:
                    emit_outcast(*pending.pop(0))

            for x in pending:
                emit_outcast(*x)

    nc.finalize()
    return nc


def _make_in_maps(cosine: np.ndarray, label: np.ndarray):
    # uint8 staging: q = round(255*x). x in [0,1) so 255*x+0.5 in [0.5,255.5)
    # and the float->int truncation implements round-half-up exactly.
    q = (cosine * np.float32(255.0) + np.float32(0.5)).astype(np.uint8)
    q3 = q.reshape(B, C, K)
    pall = np.empty((B, PALL_BYTES), dtype=np.uint8)
    for (w, route), bo, co in zip(PLAN, BYTEOFF, CHOFF):
        blk = q3[:, co : co + w, :]
        if route == "L":
            # A = (q0<<8)|q2, B = (q1<<8)|q2 little-endian: bytes [q2, qk]
            pall[:, bo : bo + 2 * w : 2] = blk[:, :, 2]
            pall[:, bo + 1 : bo + 2 * w : 2] = blk[:, :, 0]
            pall[:, bo + 2 * w : bo + 4 * w : 2] = blk[:, :, 2]
            pall[:, bo + 2 * w + 1 : bo + 4 * w : 2] = blk[:, :, 1]
        else:
            for k in range(K):
                pall[:, bo + k * w : bo + (k + 1) * w] = blk[:, :, k]
    in_maps = []
    for i in range(NCORES):
        rs = slice(i * RB, (i + 1) * RB)
        in_maps.append({"pall": np.ascontiguousarray(pall[rs])})
    return in_maps


def _postprocess(per_core_outs, cosine: np.ndarray, label: np.ndarray) -> np.ndarray:
    out_q = np.concatenate([np.asarray(o) for o in per_core_outs], axis=0)
    # dequantize + the *32 scale in one fused host multiply
    out = out_q.astype(np.float32) * np.float32(SCALE / 255.0)
    # label column: exact phi from the full-precision gathered candidates
    lab = np.asarray(label, dtype=np.int64)
    rows = np.arange(B)
    idx = (3 * lab)[:, None] + np.arange(K)[None, :]
    g3 = np.asarray(cosine, dtype=np.float32)[rows[:, None], idx]
    cl = g3.max(axis=1)
    sine = np.sqrt(np.maximum(np.float32(1.0) - cl * cl, np.float32(0.0)))
    phi = cl * np.float32(COS_M) - sine * np.float32(SIN_M)
    phi = np.where(cl > np.float32(TH), phi, cl - np.float32(MMM))
    out[rows, lab] = np.float32(SCALE) * phi.astype(np.float32)
    return np.ascontiguousarray(out)


def kernel(cosine: np.ndarray, label: np.ndarray) -> np.ndarray:
    global _CACHED_NC
    cosine = np.asarray(cosine)
    label = np.asarray(label)
    assert cosine.shape == (B, CK), cosine.shape
    assert label.shape == (B,), label.shape

    if _CACHED_NC is None:
        _CACHED_NC = build()
    nc = _CACHED_NC

    in_maps = _make_in_maps(cosine, label)
    res = run_bass_kernel_spmd(nc, in_maps, core_ids=list(range(NCORES)))
    return _postprocess(
        [res.results[i]["out"] for i in range(NCORES)], cosine, label
    )


# revision 34
# speedup vs baseline: 1.0365x; 1.0034x over previous
"""ArcMarginProduct (subcenter + inter-topk) Trainium2 kernel.

Math note: the reference uses mp=0.0, so phi_mp = cos*cos(0) + sine*sin(0)
== cos bitwise. The inter-topk term therefore cancels exactly:
    one_hot*phi + tk*phi_mp + (1-one_hot-tk)*cos == one_hot*phi + (1-one_hot)*cos
The kernel computes, per row r and class c:
    out[r, c] = 32 * max(cosine[r, 3c:3c+3])            for c != label[r]
The label column is patched on the host: the host already gathers the 3
candidate f32 values per row to stage them, and computes
out[r, l] = 32 * phi(max of the 3) directly (1024 values, postprocess).

Sharding: batch dim across 8 NeuronCores (128 rows/core = SBUF partitions).

Kernel structure (v18) — engine-balanced u8 streaming:
 - Everything moves as u8 (q = round(255*x); monotone, so max commutes;
   host dequantizes with one fused multiply).  Measured machine limits
   that shape the design:
     * DVE 2x perf mode needs EVERY operand 2-byte packed; any u8
       operand forces 1x (~1.0-1.3 ns/col vs ~0.6 at 2x).
     * total DMA ~360-430 GB/s/core across ALL queues, billed on the
       LARGER side of each transfer (so SWDGE u8->bf16 cast-loads pay
       2 B/elem); a single HWDGE queue sustains only ~300 GB/s.
     * ACT casts u8<->bf16 at ~0.92 ns/col; the GPSIMD CAST op exists
       but steals SBUF bandwidth from DVE (~3x slowdown) - unused.
 - Host stages one buffer with per-chunk route-specific layout, one
   HWDGE load per chunk (bulk loads alternate sync/sync/scalar queues).
   Per-chunk routes balance DVE, ACT, and the DMA bus:
     'L': host-packed lexicographic u16 pairs A=(q0<<8)|q2,
          B=(q1<<8)|q2; one u16 TT max at 2x gives (max(q0,q1)<<8)|q2,
          one strided-u8 byte-pair max finishes.  No casts, no SWDGE,
          fully self-contained -> used at the ramp and the tail.
     'a': ACT casts p2 u8->bf16; DVE op1 max(p0u8,p1u8)->bf16 (1x),
          op2 bf16 (2x); ACT casts the result back to u8 (exact:
          integers <= 255); HWDGE store.
     's': 'a' but the bf16 result is SWDGE cast-stored (saves ACT).
     'P': p0,p1 SWDGE cast-loaded u8->bf16 so op1 runs at 2x (DVE
          relief); ACT casts p2 in and the result out; HWDGE store.
     'p': 'P' with a SWDGE cast-store instead of the ACT out-cast.
     'u'/'q'/'g': spare routes for tuning (plain u8 chain / DVE u8 out /
          GPSIMD cast).
 - All input tiles are SBUF-resident (named) so load queues never stall
   on compute.  The compute loop is software-pipelined per chunk: ACT
   out-casts lag OLAG chunks behind the in-casts, and only the first
   V_NEARLY SWDGE cast-loads are issued upfront — the rest are chained
   behind s/p cast-stores on the gpsimd queue, so the 2-byte-billed
   SWDGE traffic stops starving the early HWDGE bulk loads.
Quantization rel err ~1.5e-3 (2e-2 gate).
"""

import math
import os
import sys

import numpy as np

os.environ.setdefault("NEURON_RT_RESET_CORES", "1")

if "/opt/trn_rl_repo" not in sys.path:
    sys.path.insert(0, "/opt/trn_rl_repo")

import concourse.bass as bass
import concourse.bacc as bacc
import concourse.mybir as mybir
from concourse.bass_utils import run_bass_kernel_spmd
from concourse.tile import TileContext

B = 1024
C = 20000          # out_features
K = 3              # subcenters
CK = C * K         # 60000
NCORES = 8
RB = B // NCORES   # 128 rows per core

# chunk plan: "<width><route>,...", see module docstring for routes
_PLAN = os.environ.get(
    "V_PLAN", "600L,1500p,2000s,2500P,2000a,2500p,2000s,2400P,2000a,1500p,1000L"
)
PLAN = [(int(t[:-1]), t[-1]) for t in _PLAN.split(",")]
assert sum(w for w, _ in PLAN) == C, PLAN
# byte offset of each chunk in the staged buffer (L chunks use 4 B/col,
# others 3 B/col)
BYTEOFF = []
_o = 0
for _w, _r in PLAN:
    BYTEOFF.append(_o)
    _o += 4 * _w if _r == "L" else 3 * _w
PALL_BYTES = _o
CHOFF = []  # column offset of each chunk
_o = 0
for _w, _r in PLAN:
    CHOFF.append(_o)
    _o += _w

SCALE = 32.0
MARGIN = 0.2
COS_M = math.cos(MARGIN)
SIN_M = math.sin(MARGIN)
TH = math.cos(math.pi - MARGIN)
MMM = 1.0 + math.cos(math.pi - MARGIN)

_CACHED_NC = None


def build():
    u8 = mybir.dt.uint8
    u16 = mybir.dt.uint16
    bf16 = mybir.dt.bfloat16
    Act = mybir.ActivationFunctionType

    nc = bacc.Bacc()
    pall_d = nc.declare_dram_parameter("pall", [RB, PALL_BYTES], u8, isOutput=False)
    out_d = nc.declare_dram_parameter("out", [RB, C], u8, isOutput=True)

    def wmax_of(routes):
        return max([w for w, r in PLAN if r in routes], default=1)

    with TileContext(nc) as tc:
        with (
            tc.tile_pool(name="bfin", bufs=1) as bfpool,
            tc.tile_pool(name="inp", bufs=3) as ipool,
            tc.tile_pool(name="lexp", bufs=2) as lpool,
            tc.tile_pool(name="mid", bufs=2) as mpool,
            tc.tile_pool(name="cast", bufs=3) as cpool,
            tc.tile_pool(name="outp", bufs=3) as opool,
            tc.tile_pool(name="pres", bufs=3) as prpool,
            tc.tile_pool(name="t01g", bufs=1) as gpool,
        ):
            # ---- gpsimd queue, part 1: SWDGE cast-loads for P/p chunks.
            # Only the first V_NEARLY are issued upfront: issuing all of
            # them floods the early DMA bus (SWDGE pays the 2B side) and
            # starves the HWDGE bulk loads DVE needs first.  The rest are
            # chained behind s/p cast-stores inside the compute loop.
            NEARLY = int(os.environ.get("V_NEARLY", "3"))
            bfin = {}
            swdge_pend = []   # chunk ids whose cast-load is deferred
            bfin_loaded = set()

            def emit_bfin_load(ci):
                w, route = PLAN[ci]
                nb = 3 * w if route == "F" else 2 * w
                nc.gpsimd.dma_start(
                    out=bfin[ci][:],
                    in_=pall_d[:, BYTEOFF[ci] : BYTEOFF[ci] + nb],
                )
                bfin_loaded.add(ci)

            for ci, (w, route) in enumerate(PLAN):
                if route in "PpqF":
                    nb = 3 * w if route == "F" else 2 * w
                    bfin[ci] = bfpool.tile([RB, nb], bf16, name=f"bfin_{ci}")
                    if len(bfin_loaded) < NEARLY:
                        emit_bfin_load(ci)
                    else:
                        swdge_pend.append(ci)

            # ---- input loads: bulk chunks alternate between the sync
            # and vector HWDGE queues (one queue sustains only ~300 GB/s);
            # small p2-only loads ride the scalar queue.
            intile = {}
            bulk_q = [{"s": nc.sync, "c": nc.scalar}[c] for c in os.environ.get("V_QPAT", "ssc")]
            bi = 0
            for ci, (w, route) in enumerate(PLAN):
                if route == "L":
                    t = bfpool.tile([RB, 4 * w], u8, name=f"lexin_{ci}")
                    bulk_q[bi % len(bulk_q)].dma_start(
                        out=t[:], in_=pall_d[:, BYTEOFF[ci] : BYTEOFF[ci] + 4 * w]
                    )
                    bi += 1
                elif route == "F":
                    t = None
                elif route in "Ppq":
                    t = bfpool.tile([RB, w], u8, name=f"p2in_{ci}")
                    nc.sync.dma_start(
                        out=t[:],
                        in_=pall_d[:, BYTEOFF[ci] + 2 * w : BYTEOFF[ci] + 3 * w],
                    )
                else:  # u/a/g/s: full 3-plane chunk
                    t = bfpool.tile([RB, 3 * w], u8, name=f"in3_{ci}")
                    bulk_q[bi % len(bulk_q)].dma_start(
                        out=t[:], in_=pall_d[:, BYTEOFF[ci] : BYTEOFF[ci] + 3 * w]
                    )
                    bi += 1
                intile[ci] = t

            def planes(ci):
                w, route = PLAN[ci]
                t = intile[ci]
                if route in "Ppq":
                    return None, None, t[:, :w]
                return t[:, :w], t[:, w : 2 * w], t[:, 2 * w : 3 * w]

            # ---- software-pipelined per-chunk emission: in-cast (ACT),
            # op1+op2 (DVE) in chunk order; ACT out-casts lag OLAG chunks
            # behind so the ACT stream interleaves in- and out-casts
            # instead of serializing all out-casts after all in-casts.
            assert not any(r == "g" for _, r in PLAN), "g route retired"
            OLAG = int(os.environ.get("V_OLAG", "2"))
            dvecast = {
                int(x) for x in os.environ.get("V_DVECAST", "").split(",") if x
            }
            bres = {}      # ci -> bf16 result tile for SWDGE cast-stores
            pending = []   # (ci, bf16 tile) awaiting ACT out-cast + store

            def emit_outcast(ci, o):
                w = PLAN[ci][0]
                o8 = opool.tile([RB, wmax_of("aPgF")], u8, tag="out8")
                nc.scalar.activation(o8[:, :w], o[:, :w], Act.Identity)
                nc.sync.dma_start(
                    out=out_d[:, CHOFF[ci] : CHOFF[ci] + w], in_=o8[:, :w]
                )

            def emit_store(ci, o):
                w = PLAN[ci][0]
                nc.sync.dma_start(
                    out=out_d[:, CHOFF[ci] : CHOFF[ci] + w], in_=o[:, :w]
                )

            for ci, (w, route) in enumerate(PLAN):
                if route in "asPpq":
                    _, _, p2 = planes(ci)
                    t = cpool.tile([RB, wmax_of("aPpgs")], bf16, tag="p2b")
                    if ci in dvecast:
                        nc.vector.tensor_copy(t[:, :w], p2)
                    else:
                        nc.scalar.activation(t[:, :w], p2, Act.Identity)
                    p2b = t

                if route == "L":
                    lex = intile[ci]
                    A = lex[:, : 2 * w].bitcast(u16)
                    Bv = lex[:, 2 * w :].bitcast(u16)
                    r = mpool.tile([RB, wmax_of("L")], u16, tag="lexr")
                    nc.vector.tensor_max(r[:, :w], A, Bv)
                    r3 = r[:, :w].bitcast(u8).rearrange("p (w k) -> p w k", k=2)
                    o = opool.tile([RB, wmax_of("L")], u8, tag="outL")
                    nc.vector.tensor_max(o[:, :w], r3[:, :, 1], r3[:, :, 0])
                    emit_store(ci, o)
                elif route == "u":
                    p0, p1, p2 = planes(ci)
                    t = mpool.tile([RB, wmax_of("u")], u8, tag="t01u")
                    nc.vector.tensor_max(t[:, :w], p0, p1)
                    o = opool.tile([RB, wmax_of("u")], u8, tag="outu")
                    nc.vector.tensor_max(o[:, :w], t[:, :w], p2)
                    emit_store(ci, o)
                elif route in "as":
                    p0, p1, _ = planes(ci)
                    t = mpool.tile([RB, wmax_of("aPpgs")], bf16, tag="t01b")
                    nc.vector.tensor_max(t[:, :w], p0, p1)
                    if route == "a":
                        o = mpool.tile([RB, wmax_of("aPgF")], bf16, tag="ob")
                        nc.vector.tensor_max(o[:, :w], t[:, :w], p2b[:, :w])
                        pending.append((ci, o))
                    else:
                        o = prpool.tile([RB, wmax_of("sp")], bf16, tag="sres")
                        nc.vector.tensor_max(o[:, :w], t[:, :w], p2b[:, :w])
                        bres[ci] = o
                elif route == "F":
                    if ci not in bfin_loaded:
                        emit_bfin_load(ci)
                        swdge_pend.remove(ci)
                    bt = bfin[ci]
                    t = mpool.tile([RB, wmax_of("aPpgs")], bf16, tag="t01b")
                    nc.vector.tensor_max(t[:, :w], bt[:, :w], bt[:, w : 2 * w])
                    o = mpool.tile([RB, wmax_of("aPgF")], bf16, tag="ob")
                    nc.vector.tensor_max(
                        o[:, :w], t[:, :w], bt[:, 2 * w : 3 * w]
                    )
                    pending.append((ci, o))
                elif route in "Ppq":
                    if ci not in bfin_loaded:
                        emit_bfin_load(ci)
                        swdge_pend.remove(ci)
                    bt = bfin[ci]
                    t = mpool.tile([RB, wmax_of("aPpgs")], bf16, tag="t01b")
                    nc.vector.tensor_max(t[:, :w], bt[:, :w], bt[:, w : 2 * w])
                    if route == "q":
                        o = opool.tile([RB, wmax_of("q")], u8, tag="outq")
                        nc.vector.tensor_max(o[:, :w], t[:, :w], p2b[:, :w])
                        emit_store(ci, o)
                    elif route == "P":
                        o = mpool.tile([RB, wmax_of("aPgF")], bf16, tag="ob")
                        nc.vector.tensor_max(o[:, :w], t[:, :w], p2b[:, :w])
                        pending.append((ci, o))
                    else:
                        o = prpool.tile([RB, wmax_of("sp")], bf16, tag="sres")
                        nc.vector.tensor_max(o[:, :w], t[:, :w], p2b[:, :w])
                        bres[ci] = o

                # s/p chunks: the SWDGE cast-store rides the gpsimd queue
                # right after this chunk's op2, and gates the next deferred
                # cast-load (naturally staggering SWDGE bus usage)
                if route in "sp":
                    nc.gpsimd.dma_start(
                        out=out_d[:, CHOFF[ci] : CHOFF[ci] + w],
                        in_=bres[ci][:, :w],
                    )
                    if swdge_pend:
                        nxt = swdge_pend.pop(0)
                        emit_bfin_load(nxt)

                while len(pending) > OLAG:
                    emit_outcast(*pending.pop(0))

            for x in pending:
                emit_outcast(*x)

    nc.finalize()
    return nc


def _make_in_maps(cosine: np.ndarray, label: np.ndarray):
    # uint8 staging: q = round(255*x). x in [0,1) so 255*x+0.5 in [0.5,255.5)
    # and the float->int truncation implements round-half-up exactly.
    q = (cosine * np.float32(255.0) + np.float32(0.5)).astype(np.uint8)
    q3 = q.reshape(B, C, K)
    pall = np.empty((B, PALL_BYTES), dtype=np.uint8)
    for (w, route), bo, co in zip(PLAN, BYTEOFF, CHOFF):
        blk = q3[:, co : co + w, :]
        if route == "L":
            # A = (q0<<8)|q2, B = (q1<<8)|q2 little-endian: bytes [q2, qk]
            pall[:, bo : bo + 2 * w : 2] = blk[:, :, 2]
            pall[:, bo + 1 : bo + 2 * w : 2] = blk[:, :, 0]
            pall[:, bo + 2 * w : bo + 4 * w : 2] = blk[:, :, 2]
            pall[:, bo + 2 * w + 1 : bo + 4 * w : 2] = blk[:, :, 1]
        else:
            for k in range(K):
                pall[:, bo + k * w : bo + (k + 1) * w] = blk[:, :, k]
    in_maps = []
    for i in range(NCORES):
        rs = slice(i * RB, (i + 1) * RB)
        in_maps.append({"pall": np.ascontiguousarray(pall[rs])})
    return in_maps


def _postprocess(per_core_outs, cosine: np.ndarray, label: np.ndarray) -> np.ndarray:
    out_q = np.concatenate([np.asarray(o) for o in per_core_outs], axis=0)
    # dequantize + the *32 scale in one fused host multiply
    out = out_q.astype(np.float32) * np.float32(SCALE / 255.0)
    # label column: exact phi from the full-precision gathered candidates
    lab = np.asarray(label, dtype=np.int64)
    rows = np.arange(B)
    idx = (3 * lab)[:, None] + np.arange(K)[None, :]
    g3 = np.asarray(cosine, dtype=np.float32)[rows[:, None], idx]
    cl = g3.max(axis=1)
    sine = np.sqrt(np.maximum(np.float32(1.0) - cl * cl, np.float32(0.0)))
    phi = cl * np.float32(COS_M) - sine * np.float32(SIN_M)
    phi = np.where(cl > np.float32(TH), phi, cl - np.float32(MMM))
    out[rows, lab] = np.float32(SCALE) * phi.astype(np.float32)
    return np.ascontiguousarray(out)


def kernel(cosine: np.ndarray, label: np.ndarray) -> np.ndarray:
    global _CACHED_NC
    cosine = np.asarray(cosine)
    label = np.asarray(label)
    assert cosine.shape == (B, CK), cosine.shape
    assert label.shape == (B,), label.shape

    if _CACHED_NC is None:
        _CACHED_NC = build()
    nc = _CACHED_NC

    in_maps = _make_in_maps(cosine, label)
    res = run_bass_kernel_spmd(nc, in_maps, core_ids=list(range(NCORES)))
    return _postprocess(
        [res.results[i]["out"] for i in range(NCORES)], cosine, label
    )
